# revision 37
# baseline (speedup 1.0000x reference)
"""Trainium2 Bass kernel for nn_Bottleneck_57561151701110 (SAM pairwise
bottleneck block). Data-parallel over batch: 8 images -> 8 NeuronCores.

Per-core pipeline (one 256x56x56 image, x kept resident in SBUF):
  conv1+conv2 (PE f32r) -> ACT bias move -> SBUF DMA band scatter ->
  pairwise feat = x1 - shift(x2) (DVE bf16 2x / GPSIMD for the odd-parity
  dw=0 shifts) -> w1/w2 blockdiag matmuls (PE bf16) + ACT relu/exp ->
  Z = sum_k e_k on GPSIMD, 1/Z via fast-approx reciprocal (DVE) ->
  aggregation: 9 broadcast multiplies (DVE bf16 2x, 2/round on GPSIMD)
  + PE identity-accumulate into PSUM -> leaky + 1/Z scale -> SBUF DMA
  channel scatter -> conv_out (PE bf16) -> leaky + residual -> out.

BN layers are folded into conv weights/biases on the host. The position
branch enters the w1 matmul as an extra low-rank accumulate. All DMA
triggers issue from the GPSIMD queue (cheap dispatch); x1/x2/x3/sam
partition reshuffles are SBUF->SBUF DMAs (no DRAM roundtrips).
"""

import os
import sys

for _p in ("/opt/trn_rl_repo", os.path.expanduser("~/.axon_site/_ro/trn_rl_repo")):
    if os.path.isdir(_p) and _p not in sys.path:
        sys.path.insert(0, _p)

from contextlib import ExitStack

import numpy as np

import concourse.bass as bass
import concourse.bacc as bacc
import concourse.tile as tile
from concourse import mybir
from concourse.bass_utils import run_bass_kernel_spmd

dt = mybir.dt
ALU = mybir.AluOpType
ACTF = mybir.ActivationFunctionType

B, CIN, H, W = 8, 256, 56, 56
NPIX = H * W            # 3136
REL, MID, OUT = 32, 256, 256
SHARE = 8
NB = 4                  # row bands
BH = H // NB            # 14 rows per band
Q = BH * W              # 784 band pixels
BR = BH + 2             # 16 padded band rows
HBW = Q // 2            # 392 half-band pixels
NEG = 0.01
BN_EPS = 1e-5
CCH = 448               # conv free chunk (8 rows)
NCH = NPIX // CCH       # 7
OFFS = [(dh, dw) for dh in (-1, 0, 1) for dw in (-1, 0, 1)]

F32, F32R, BF16 = dt.float32, dt.float32r, dt.float16  # 16-bit = fp16

_CACHE = {}


def _band_rows(b):
    # image-row window covered by band b's x2/x3 tiles (halo included)
    return max(14 * b - 1, 0), min(14 * b + 15, 56)


# ----------------------------------------------------------------- host prep
def _position(h, w):
    loc_w = np.tile(np.linspace(-1.0, 1.0, w, dtype=np.float32)[None, :], (h, 1))
    loc_h = np.tile(np.linspace(-1.0, 1.0, h, dtype=np.float32)[:, None], (1, w))
    return np.stack([loc_w, loc_h], axis=0)  # (2, H, W)


def _host_consts(inp):
    f32 = np.float32
    inv_a = (inp["bna_g"] / np.sqrt(inp["bna_v"] + BN_EPS)).astype(f32)
    beta_a = (inp["bna_b"] - inp["bna_m"] * inv_a).astype(f32)
    inv_b = (inp["bnb_g"] / np.sqrt(inp["bnb_v"] + BN_EPS)).astype(f32)
    beta_b = (inp["bnb_b"] - inp["bnb_m"] * inv_b).astype(f32)

    w1c = inp["conv1_w"] * inv_a[:REL, None]
    b1 = inp["conv1_b"] * inv_a[:REL] + beta_a[:REL]
    w2c = inp["conv2_w"] * inv_a[:REL, None]
    b2 = inp["conv2_b"] * inv_a[:REL]

    # conv1+conv2 fused stationary: (256, 64), chunked over K
    c12 = np.concatenate([w1c, w2c], axis=0).T.astype(f32).copy()  # (256,64)

    c3 = inp["conv3_w"].T.astype(f32).copy()    # (256,256) lhsT
    # conv_out lhsT with contraction rows permuted to the aggregation's
    # scatter-friendly order: row 128t+r holds input channel
    # 8*(16t + gl) + 4*sq + a where r = (sq, gl, a) = (r>>6, (r&63)>>2, r&3)
    co_n = inp["convo_w"].T.astype(f32)         # (256 in, 256 out)
    co = np.zeros_like(co_n)
    for t in range(2):
        for r in range(128):
            sq, gl, a = r >> 6, (r & 63) >> 2, r & 3
            co[128 * t + r] = co_n[8 * (16 * t + gl) + 4 * sq + a]

    # W1' with bnb scale folded
    w1p = (inp["w1"] * inv_b[:, None]).astype(f32)  # (32, 34)
    w1a, w1b = w1p[:, :REL], w1p[:, REL:]
    lhsT_w1 = np.zeros((128, 128), f32)
    lhsT_pos = np.zeros((8, 128), f32)
    lhsT_w2 = np.zeros((128, 128), f32)
    for b in range(NB):
        lhsT_w1[32 * b:32 * b + 32, 32 * b:32 * b + 32] = w1a.T  # [c, o]
        lhsT_pos[2 * b:2 * b + 2, 32 * b:32 * b + 32] = w1b.T    # [c2, o]
        lhsT_w2[32 * b:32 * b + 32, 32 * b:32 * b + 32] = inp["w2"].T  # [o, g]

    betab = np.tile(beta_b, NB).astype(f32).reshape(128)
    w2bv = np.tile(inp["w2_b"], NB).astype(f32).reshape(128)

    # position branch, batch independent: posr[2b+c2, 784k+q] =
    # relu(inv_a[32+c2]*subp[c2,k,band b pix q] + beta_a[32+c2])
    pos = _position(H, W)
    pc = np.einsum("oc,chw->ohw", inp["convp_w"], pos) + inp["convp_b"][:, None, None]
    pcp = np.pad(pc, ((0, 0), (1, 1), (1, 1)))
    posr = np.zeros((8, 9 * Q), f32)
    for k, (dh, dw) in enumerate(OFFS):
        sub = pc - pcp[:, 1 + dh:1 + dh + H, 1 + dw:1 + dw + W]  # (2,56,56)
        v = np.maximum(inv_a[REL:, None, None] * sub + beta_a[REL:, None, None], 0.0)
        vb = v.reshape(2, NB, BH, W)  # (c2, b, r, w)
        for b in range(NB):
            posr[2 * b:2 * b + 2, Q * k:Q * (k + 1)] = vb[:, b].reshape(2, Q)

    vecs = np.zeros((128, 12), f32)
    vecs[:REL, 0] = b1
    vecs[REL:2 * REL, 0] = b2
    vecs[:, 1] = inp["conv3_b"][:128]
    vecs[:, 2] = inp["conv3_b"][128:]
    vecs[:, 3] = betab
    vecs[:, 4] = w2bv
    vecs[:, 5] = -(1.0 - NEG) * inp["convo_b"][:128]
    vecs[:, 6] = -(1.0 - NEG) * inp["convo_b"][128:]
    vecs[:, 7] = inp["convo_b"][:128]
    vecs[:, 8] = inp["convo_b"][128:]

    bf16 = np.float16

    def kchunked(a):
        # (256, N) lhsT -> [128, 2, N]: partition p holds rows p and 128+p
        n = a.shape[1]
        return np.ascontiguousarray(a.reshape(2, 128, n).transpose(1, 0, 2))

    def flat(a):
        return np.ascontiguousarray(a).reshape(128, -1)

    # packed per-dtype const blocks, one DMA each (offsets in _build)
    constsAf = np.concatenate([
        flat(kchunked(c12).astype(f32)),             # 128 w
        flat(kchunked(c3).astype(f32)),              # 512 w
        flat(np.eye(128, dtype=f32)),                # 128 w
        flat(vecs),                                  # 12 w
    ], axis=1)
    constsAb = np.concatenate([
        flat(kchunked(co).astype(bf16)),             # 512 e
        flat(lhsT_w1.astype(bf16)),                  # 128 e
        flat(lhsT_w2.astype(bf16)),                  # 128 e
        flat(np.eye(128, dtype=bf16)),               # 128 e
    ], axis=1)
    # 8-partition consts: [8, 128 + 9*784] bf16 = (lhsT_pos | posr)
    constsB = np.concatenate([lhsT_pos.astype(bf16), posr.astype(bf16)],
                             axis=1)
    return {"constsAf": constsAf, "constsAb": constsAb, "constsB": constsB}


# ------------------------------------------------------------ program build
def _build_program():
    nc = bacc.Bacc("TRN2", target_bir_lowering=False, debug=False,
                   enable_asserts=False, num_devices=8)

    def dram_in(name, shape, dtype):
        return nc.dram_tensor(name, list(shape), dtype, kind="ExternalInput").ap()

    xin = dram_in("xin", (CIN, NPIX), F32R)
    cAfd = dram_in("constsAf", (128, 780), F32R)
    cAbd = dram_in("constsAb", (128, 896), BF16)
    cBd = dram_in("constsB", (8, 128 + 9 * Q), BF16)

    outd = nc.dram_tensor("out", [CIN, NPIX], F32, kind="ExternalOutput").ap()

    with tile.TileContext(nc) as tc, ExitStack() as ctx:
        nc_ = tc.nc
        sdma = nc.sync.dma_start            # HW DGE queue (SP). The Pool
                                            # queue's SW DGE burns ~1us/DMA
                                            # of GPSIMD engine time.

        cpool = ctx.enter_context(tc.tile_pool(name="consts", bufs=1))
        xpool = ctx.enter_context(tc.tile_pool(name="xres", bufs=1))
        bpool = ctx.enter_context(tc.tile_pool(name="bands", bufs=1))
        fpool = ctx.enter_context(tc.tile_pool(name="fpool", bufs=4))
        hpool = ctx.enter_context(tc.tile_pool(name="hpool", bufs=2))
        epool = ctx.enter_context(tc.tile_pool(name="epool", bufs=9))
        zpool = ctx.enter_context(tc.tile_pool(name="zpool", bufs=1))
        pkpool = ctx.enter_context(tc.tile_pool(name="pkpool", bufs=12))
        rqpool = ctx.enter_context(tc.tile_pool(name="rqpool", bufs=2))
        sqpool = ctx.enter_context(tc.tile_pool(name="sqpool", bufs=2))
        smcpool = ctx.enter_context(tc.tile_pool(name="smcpool", bufs=1))
        opool = ctx.enter_context(tc.tile_pool(name="opool", bufs=4))

        # ---- const loads: three packed DMAs split across both HW DGE
        # queues (sync + scalar) so the front-door transfers run in parallel
        adma = nc.scalar.dma_start
        cAf = cpool.tile([128, 780], F32R, tag="cAf")
        sdma(cAf[:], cAfd[:])
        cAb = cpool.tile([128, 896], BF16, tag="cAb")
        adma(cAb[:], cAbd[:])
        cB = cpool.tile([8, 128 + 9 * Q], BF16, tag="cB")
        adma(cB[:], cBd[:])
        c12t = cAf[:, 0:128].rearrange("p (a b) -> p a b", a=2)
        c3t = cAf[:, 128:640].rearrange("p (a b) -> p a b", a=2)
        identft = cAf[:, 640:768]
        vecst = cAf[:, 768:780].bitcast(F32)
        cot = cAb[:, 0:512].rearrange("p (a b) -> p a b", a=2)
        w1t = cAb[:, 512:640]
        w2t = cAb[:, 640:768]
        identt = cAb[:, 768:896]
        post = cB[:, 0:128]
        prts = [cB[:, 128 + Q * k:128 + Q * (k + 1)] for k in range(9)]

        # ---- x resident in SBUF, loaded in row groups (sync queue)
        xt = [xpool.tile([128, NPIX], F32R, tag=f"xt{t}", name=f"xt{t}")
              for t in range(2)]
        for (rl, rh) in ((0, 16), (16, 30), (30, 44), (44, 56)):
            sdma(xt[0][:, rl * W:rh * W], xin[0:128, rl * W:rh * W])
            adma(xt[1][:, rl * W:rh * W], xin[128:256, rl * W:rh * W])

        # ---- band tiles + halo memsets
        x12s = bpool.tile([64, NPIX], BF16, tag="x12s")
        x1b = bpool.tile([128, BH, W], BF16, tag="x1b")
        x2b = bpool.tile([128, BR, 58], BF16, tag="x2b")
        xgA = bpool.tile([128, SHARE, BR, 58], BF16, tag="xgA")
        xgB = bpool.tile([128, SHARE, BR, 58], BF16, tag="xgB")
        nc_.gpsimd.memset(x2b[:], 0.0)
        for xg_ in (xgA, xgB):
            nc_.gpsimd.memset(xg_[0:32, :, 0:1, :], 0.0)
            nc_.gpsimd.memset(xg_[96:128, :, 15:16, :], 0.0)

        # ---- conv1+conv2: 7 chunks, ACT moves bias into x12s
        pscopeA = ExitStack()
        pp12 = pscopeA.enter_context(tc.tile_pool(name="pp12", bufs=3, space="PSUM"))
        for c in range(NCH):
            sl = slice(CCH * c, CCH * (c + 1))
            ps = pp12.tile([64, CCH], F32, tag="ps12")
            nc_.tensor.matmul(ps[:], c12t[:, 0, :], xt[0][:, sl],
                              start=True, stop=False)
            nc_.tensor.matmul(ps[:], c12t[:, 1, :], xt[1][:, sl],
                              start=False, stop=True)
            nc_.scalar.activation(x12s[:, sl], ps[:], ACTF.Identity,
                                  bias=vecst[0:64, 0:1])
        pscopeA.close()

        # ---- x1/x2 band scatter (SBUF->SBUF, vector queue: self-gating
        # with the DVE feat ops that consume them)
        x1bf = x1b[:].rearrange("p r w -> p (r w)")
        for b in range(NB):
            lo, hi = _band_rows(b)
            r0 = lo - (14 * b - 1)
            sdma(x1bf[32 * b:32 * b + 32, :], x12s[0:32, Q * b:Q * (b + 1)])
            sdma(x2b[32 * b:32 * b + 32, r0:r0 + (hi - lo), 1:57],
                 x12s[32:64, lo * W:hi * W].rearrange("p (r w) -> p r w", w=W))

        # ---- conv3 -> per-t staging tile -> xgA/xgB band scatter
        x3a0 = bpool.tile([128, H, 58], BF16, tag="x3a0")
        x3a1 = bpool.tile([128, H, 58], BF16, tag="x3a1")
        x3all = [x3a0, x3a1]
        for t in range(2):
            nc_.gpsimd.memset(x3all[t][:, :, 0:1], 0.0)
            nc_.gpsimd.memset(x3all[t][:, :, 57:58], 0.0)
        # band b is fully staged once chunk BLAST[b] is written
        BLAST = {1: 0, 3: 1, 5: 2, 6: 3}
        pscopeB = ExitStack()
        pp3 = pscopeB.enter_context(tc.tile_pool(name="pp3", bufs=3, space="PSUM"))
        for c in range(NCH):
            for t in range(2):
                ps3 = pp3.tile([128, CCH], F32, tag="ps3")
                nc_.tensor.matmul(ps3[:], c3t[:, 0, 128 * t:128 * (t + 1)],
                                  xt[0][:, CCH * c:CCH * (c + 1)],
                                  start=True, stop=False)
                nc_.tensor.matmul(ps3[:], c3t[:, 1, 128 * t:128 * (t + 1)],
                                  xt[1][:, CCH * c:CCH * (c + 1)],
                                  start=False, stop=True)
                nc_.scalar.activation(x3all[t][:, 8 * c:8 * c + 8, 1:57],
                                      ps3[:].rearrange("p (r w) -> p r w", w=W),
                                      ACTF.Identity, bias=vecst[:, 1 + t:2 + t])
            if c in BLAST:
                b = BLAST[c]
                lo, hi = _band_rows(b)
                dr0 = lo - (14 * b - 1)
                nrr = hi - lo
                for t in range(2):
                    x3f = x3all[t][:].rearrange("p r w -> p (r w)")
                    psl = slice(32 * b + 16 * t, 32 * b + 16 * t + 16)
                    dstA = xgA[psl].rearrange("p s r w -> p s (r w)")
                    sdma(dstA[:, :, dr0 * 58:(dr0 + nrr) * 58],
                         x3f[:, lo * 58:hi * 58])
                    # xgB[w] = xgA[w+1]: shifted flat copy, 1 desc/partition
                    dstB = xgB[psl].rearrange("p s r w -> p s (r w)")
                    sdma(dstB[:, :, dr0 * 58:(dr0 + nrr) * 58 - 1],
                         x3f[:, lo * 58 + 1:hi * 58])
        pscopeB.close()

        # ---- phase C: per-k logits -> exp; Z accumulated on GPSIMD
        wsl = ((0, 512), (512, 272))
        pscopeC = ExitStack()
        pph = pscopeC.enter_context(tc.tile_pool(name="pph", bufs=3, space="PSUM"))
        ek = []
        zs = [zpool.tile([128, Q], F32, tag=f"z{i}", name=f"z{i}")
              for i in range(2)]
        zcur = None
        for k, (dh, dw) in enumerate(OFFS):
            win = x2b[:, 1 + dh:15 + dh, 1 + dw:57 + dw]
            ftmp = fpool.tile([128, BH, W], BF16, tag="ftmp")
            if dw == 0:
                nc_.gpsimd.tensor_tensor(ftmp[:], x1b[:], win, ALU.subtract)
            else:
                nc_.vector.tensor_tensor(ftmp[:], x1b[:], win, ALU.subtract)
            fr = fpool.tile([128, BH, W], BF16, tag="fr")
            nc_.vector.tensor_scalar(fr[:], ftmp[:], 0.0, None, op0=ALU.max)
            frf = fr[:].rearrange("p r w -> p (r w)")
            hps = pph.tile([128, 1024], F32, tag="hw")
            for (o0, nn) in wsl:
                nc_.tensor.matmul(hps[:, o0:o0 + nn], w1t[:], frf[:, o0:o0 + nn],
                                  start=True, stop=False)
                nc_.tensor.matmul(hps[:, o0:o0 + nn], post[:],
                                  prts[k][:, o0:o0 + nn], start=False, stop=True)
            hp = hpool.tile([128, Q], BF16, tag="hp")
            nc_.scalar.activation(hp[:], hps[:, 0:Q], ACTF.Relu,
                                  bias=vecst[:, 3:4])
            wps = pph.tile([128, 1024], F32, tag="hw")
            for (o0, nn) in wsl:
                nc_.tensor.matmul(wps[:, o0:o0 + nn], w2t[:], hp[:, o0:o0 + nn],
                                  start=True, stop=True)
            e = epool.tile([128, Q], BF16, tag="e")
            nc_.scalar.activation(e[:], wps[:, 0:Q], ACTF.Exp,
                                  bias=vecst[:, 4:5])
            ek.append(e)
            if k == 1:
                nc_.gpsimd.tensor_tensor(zs[0][:], ek[0][:], ek[1][:], ALU.add)
                zcur = 0
            elif k >= 2:
                nc_.gpsimd.tensor_tensor(zs[1 - zcur][:], e[:], zs[zcur][:],
                                         ALU.add)
                zcur = 1 - zcur
        pscopeC.close()

        # ---- aggregation rounds (qp: row half, sq: s-quad), software-
        # pipelined: round r+1's DVE products are emitted before round r's
        # post so the PE identity stream never stalls at round boundaries.
        pscopeE = ExitStack()
        pps = pscopeE.enter_context(tc.tile_pool(name="pps", bufs=1, space="PSUM"))
        ppo = pscopeE.enter_context(tc.tile_pool(name="ppo", bufs=2, space="PSUM"))
        korder = [2, 3, 4, 5, 6, 7, 8, 0, 1]
        rounds = [(0, 0), (0, 1), (1, 0), (1, 1)]
        smcs = {}
        for qp in range(2):
            for bb in range(NB):
                for t in range(2):
                    smcs[(qp, bb, t)] = smcpool.tile(
                        [128, HBW], BF16, tag=f"smc{qp}_{bb}_{t}",
                        name=f"smc{qp}_{bb}_{t}")
        pks = {}
        sams = {}

        def emit_products(ri):
            qp, sq = rounds[ri]
            lst = []
            for k in korder:
                dh, dw = OFFS[k]
                xgt = xgB if dw == 0 else xgA
                co_ = 0 if dw == 0 else 1 + dw
                r0 = 1 + dh + 7 * qp
                pk = pkpool.tile([128, 4, 7, W], BF16, tag="pk", name="pk")
                win = xgt[:, 4 * sq:4 * sq + 4, r0:r0 + 7, co_:co_ + W]
                ein = (ek[k][:].rearrange("p (r w) -> p r w", w=W)
                       [:, 7 * qp:7 * qp + 7, :].unsqueeze(1)
                       .broadcast_to((128, 4, 7, W)))
                nc_.vector.tensor_tensor(pk[:], win, ein, ALU.mult)
                lst.append(pk)
            pks[ri] = lst

        def emit_mms(ri):
            sam = pps.tile([128, 2048], F32, tag="sam", name="sam")
            sams[ri] = sam
            for j, pk in enumerate(pks[ri]):
                pkf = pk[:].rearrange("p a r w -> p (a r w)")
                for a in range(4):
                    nc_.tensor.matmul(
                        sam[:, 512 * a:512 * a + HBW], identt[:],
                        pkf[:, HBW * a:HBW * (a + 1)],
                        start=(j == 0), stop=(j == 8))

        def emit_post(ri):
            qp, sq = rounds[ri]
            sam = sams[ri]
            rq = rqpool.tile([128, 4, HBW], BF16, tag="rq", name="rq")
            sq_ = sqpool.tile([128, 4, HBW], BF16, tag="sq", name="sq_")
            for a in range(4):
                nc_.scalar.activation(rq[:, a, :],
                                      sam[:, 512 * a:512 * a + HBW],
                                      ACTF.Relu, scale=-(1.0 - NEG))
                nc_.vector.scalar_tensor_tensor(
                    sq_[:, a, :], sam[:, 512 * a:512 * a + HBW], 1.0,
                    rq[:, a, :], ALU.mult, ALU.add)
                nc_.vector.tensor_tensor(
                    sq_[:, a, :], sq_[:, a, :],
                    rz16[:, HBW * qp:HBW * (qp + 1)], ALU.mult)
            # scatter to conv_out rhs layout (SBUF->SBUF): smc partition
            # 64sq+4gl+a <- src partition 32bb+16t+gl lane a (the convo
            # lhsT rows are host-permuted to match)
            for bb in range(NB):
                for t in range(2):
                    p0 = 32 * bb + 16 * t
                    sdma(smcs[(qp, bb, t)][64 * sq:64 * sq + 64, :],
                         sq_[p0:p0 + 16, :, :])

        def emit_convo(qp):
            # conv_out + leaky + residual; the leaky correction
            # r = relu(-0.99(p+b)) and the residual x are identity-
            # accumulated into PSUM so DVE stays out of the tail:
            # out = p + x + r + b = leaky(p + b) + x
            for bb in range(NB):
                po = (14 * bb + 7 * qp) * W
                for tp in range(2):
                    pso = ppo.tile([128, HBW], F32, tag="pso", name="pso")
                    nc_.tensor.matmul(pso[:], cot[:, 0, 128 * tp:128 * (tp + 1)],
                                      smcs[(qp, bb, 0)][:],
                                      start=True, stop=False)
                    nc_.tensor.matmul(pso[:], cot[:, 1, 128 * tp:128 * (tp + 1)],
                                      smcs[(qp, bb, 1)][:],
                                      start=False, stop=True)
                    r = opool.tile([128, HBW], BF16, tag="r", name="r")
                    nc_.scalar.activation(r[:], pso[:], ACTF.Relu,
                                          scale=-(1.0 - NEG),
                                          bias=vecst[:, 5 + tp:6 + tp])
                    # keep accumulating into the (hw-wise still live) psum:
                    # stop above only closes the sim's group bookkeeping
                    nc_.tensor.matmul(pso[:], identft[:],
                                      xt[tp][:, po:po + HBW],
                                      start=False, stop=False,
                                      skip_group_check=True)
                    nc_.tensor.matmul(pso[:], identt[:], r[:],
                                      start=False, stop=True,
                                      skip_group_check=True)
                    o = opool.tile([128, HBW], F32, tag="o", name="o")
                    nc_.scalar.activation(o[:], pso[:], ACTF.Identity,
                                          bias=vecst[:, 7 + tp:8 + tp])
                    sdma(outd[128 * tp:128 * (tp + 1), po:po + HBW], o[:])

        emit_products(0)
        emit_mms(0)
        # 1/Z (applied after leaky: positively homogeneous); emitted after
        # round-0 products so the DVE never idles waiting on the GPSIMD Z
        rz16 = zpool.tile([128, Q], BF16, tag="rz16")
        nc_.vector.reciprocal_approx_fast(zs[1 - zcur][:], zs[zcur][:])
        nc_.vector.tensor_copy(rz16[:], zs[1 - zcur][:])

        emit_products(1)
        emit_post(0)
        emit_mms(1)
        emit_products(2)
        emit_post(1)
        emit_mms(2)
        emit_convo(0)
        emit_products(3)
        emit_post(2)
        emit_mms(3)
        emit_post(3)
        emit_convo(1)
        pscopeE.close()

    nc.compile()
    return nc


# --------------------------------------------------------------- entrypoint
def _get_program():
    if "nc" not in _CACHE:
        _CACHE["nc"] = _build_program()
    return _CACHE["nc"]


def _in_maps(inputs):
    consts = _host_consts(inputs)
    x = inputs["x"].reshape(B, CIN, NPIX).astype(np.float32)
    in_maps = []
    for b in range(B):
        m = {k: v for k, v in consts.items()}
        m["xin"] = x[b]
        in_maps.append(m)
    return in_maps


def kernel(**inputs):
    inputs = {k: np.asarray(v) for k, v in inputs.items()}
    nc = _get_program()
    res = run_bass_kernel_spmd(nc, _in_maps(inputs), list(range(B)))
    out = np.stack([res.results[i]["out"] for i in range(B)])
    return out.reshape(B, CIN, H, W).astype(np.float32)


def kernel_traced(**inputs):
    """Like kernel() but with NTFF tracing; returns (out, BassKernelResults)."""
    inputs = {k: np.asarray(v) for k, v in inputs.items()}
    nc = _get_program()
    res = run_bass_kernel_spmd(nc, _in_maps(inputs), list(range(B)), trace=True)
    out = np.stack([res.results[i]["out"] for i in range(B)])
    return out.reshape(B, CIN, H, W).astype(np.float32), res


# revision 40
# speedup vs baseline: 1.0000x; 1.0000x over previous
"""Trainium2 Bass kernel for nn_Bottleneck_57561151701110 (SAM pairwise
bottleneck block). Data-parallel over batch: 8 images -> 8 NeuronCores.

Per-core pipeline (one 256x56x56 image, x kept resident in SBUF):
  conv1+conv2 (PE f32r) -> ACT bias move -> SBUF DMA band scatter ->
  pairwise feat = x1 - shift(x2) (DVE bf16 2x / GPSIMD for the odd-parity
  dw=0 shifts) -> w1/w2 blockdiag matmuls (PE bf16) + ACT relu/exp ->
  Z = sum_k e_k on GPSIMD, 1/Z via fast-approx reciprocal (DVE) ->
  aggregation: 9 broadcast multiplies (DVE bf16 2x, 2/round on GPSIMD)
  + PE identity-accumulate into PSUM -> leaky + 1/Z scale -> SBUF DMA
  channel scatter -> conv_out (PE bf16) -> leaky + residual -> out.

BN layers are folded into conv weights/biases on the host. The position
branch enters the w1 matmul as an extra low-rank accumulate. All DMA
triggers issue from the GPSIMD queue (cheap dispatch); x1/x2/x3/sam
partition reshuffles are SBUF->SBUF DMAs (no DRAM roundtrips).
"""

import os
import sys

for _p in ("/opt/trn_rl_repo", os.path.expanduser("~/.axon_site/_ro/trn_rl_repo")):
    if os.path.isdir(_p) and _p not in sys.path:
        sys.path.insert(0, _p)

from contextlib import ExitStack

import numpy as np

import concourse.bass as bass
import concourse.bacc as bacc
import concourse.tile as tile
from concourse import mybir
from concourse.bass_utils import run_bass_kernel_spmd

dt = mybir.dt
ALU = mybir.AluOpType
ACTF = mybir.ActivationFunctionType

B, CIN, H, W = 8, 256, 56, 56
NPIX = H * W            # 3136
REL, MID, OUT = 32, 256, 256
SHARE = 8
NB = 4                  # row bands
BH = H // NB            # 14 rows per band
Q = BH * W              # 784 band pixels
BR = BH + 2             # 16 padded band rows
HBW = Q // 2            # 392 half-band pixels
NEG = 0.01
BN_EPS = 1e-5
CCH = 448               # conv free chunk (8 rows)
NCH = NPIX // CCH       # 7
OFFS = [(dh, dw) for dh in (-1, 0, 1) for dw in (-1, 0, 1)]

F32, F32R, BF16 = dt.float32, dt.float32r, dt.float16  # 16-bit = fp16

_CACHE = {}


def _band_rows(b):
    # image-row window covered by band b's x2/x3 tiles (halo included)
    return max(14 * b - 1, 0), min(14 * b + 15, 56)


# ----------------------------------------------------------------- host prep
def _position(h, w):
    loc_w = np.tile(np.linspace(-1.0, 1.0, w, dtype=np.float32)[None, :], (h, 1))
    loc_h = np.tile(np.linspace(-1.0, 1.0, h, dtype=np.float32)[:, None], (1, w))
    return np.stack([loc_w, loc_h], axis=0)  # (2, H, W)


def _host_consts(inp):
    f32 = np.float32
    inv_a = (inp["bna_g"] / np.sqrt(inp["bna_v"] + BN_EPS)).astype(f32)
    beta_a = (inp["bna_b"] - inp["bna_m"] * inv_a).astype(f32)
    inv_b = (inp["bnb_g"] / np.sqrt(inp["bnb_v"] + BN_EPS)).astype(f32)
    beta_b = (inp["bnb_b"] - inp["bnb_m"] * inv_b).astype(f32)

    w1c = inp["conv1_w"] * inv_a[:REL, None]
    b1 = inp["conv1_b"] * inv_a[:REL] + beta_a[:REL]
    w2c = inp["conv2_w"] * inv_a[:REL, None]
    b2 = inp["conv2_b"] * inv_a[:REL]

    # conv1+conv2 fused stationary: (256, 64), chunked over K
    c12 = np.concatenate([w1c, w2c], axis=0).T.astype(f32).copy()  # (256,64)

    c3 = inp["conv3_w"].T.astype(f32).copy()    # (256,256) lhsT
    # conv_out lhsT with contraction rows permuted to the aggregation's
    # scatter-friendly order: row 128t+r holds input channel
    # 8*(16t + gl) + 4*sq + a where r = (sq, gl, a) = (r>>6, (r&63)>>2, r&3)
    co_n = inp["convo_w"].T.astype(f32)         # (256 in, 256 out)
    co = np.zeros_like(co_n)
    for t in range(2):
        for r in range(128):
            sq, gl, a = r >> 6, (r & 63) >> 2, r & 3
            co[128 * t + r] = co_n[8 * (16 * t + gl) + 4 * sq + a]

    # W1' with bnb scale folded
    w1p = (inp["w1"] * inv_b[:, None]).astype(f32)  # (32, 34)
    w1a, w1b = w1p[:, :REL], w1p[:, REL:]
    lhsT_w1 = np.zeros((128, 128), f32)
    lhsT_pos = np.zeros((8, 128), f32)
    lhsT_w2 = np.zeros((128, 128), f32)
    for b in range(NB):
        lhsT_w1[32 * b:32 * b + 32, 32 * b:32 * b + 32] = w1a.T  # [c, o]
        lhsT_pos[2 * b:2 * b + 2, 32 * b:32 * b + 32] = w1b.T    # [c2, o]
        lhsT_w2[32 * b:32 * b + 32, 32 * b:32 * b + 32] = inp["w2"].T  # [o, g]

    betab = np.tile(beta_b, NB).astype(f32).reshape(128)
    w2bv = np.tile(inp["w2_b"], NB).astype(f32).reshape(128)

    # position branch, batch independent: posr[2b+c2, 784k+q] =
    # relu(inv_a[32+c2]*subp[c2,k,band b pix q] + beta_a[32+c2])
    pos = _position(H, W)
    pc = np.einsum("oc,chw->ohw", inp["convp_w"], pos) + inp["convp_b"][:, None, None]
    pcp = np.pad(pc, ((0, 0), (1, 1), (1, 1)))
    posr = np.zeros((8, 9 * Q), f32)
    for k, (dh, dw) in enumerate(OFFS):
        sub = pc - pcp[:, 1 + dh:1 + dh + H, 1 + dw:1 + dw + W]  # (2,56,56)
        v = np.maximum(inv_a[REL:, None, None] * sub + beta_a[REL:, None, None], 0.0)
        vb = v.reshape(2, NB, BH, W)  # (c2, b, r, w)
        for b in range(NB):
            posr[2 * b:2 * b + 2, Q * k:Q * (k + 1)] = vb[:, b].reshape(2, Q)

    vecs = np.zeros((128, 12), f32)
    vecs[:REL, 0] = b1
    vecs[REL:2 * REL, 0] = b2
    vecs[:, 1] = inp["conv3_b"][:128]
    vecs[:, 2] = inp["conv3_b"][128:]
    vecs[:, 3] = betab
    vecs[:, 4] = w2bv
    vecs[:, 5] = -(1.0 - NEG) * inp["convo_b"][:128]
    vecs[:, 6] = -(1.0 - NEG) * inp["convo_b"][128:]
    vecs[:, 7] = inp["convo_b"][:128]
    vecs[:, 8] = inp["convo_b"][128:]

    bf16 = np.float16

    def kchunked(a):
        # (256, N) lhsT -> [128, 2, N]: partition p holds rows p and 128+p
        n = a.shape[1]
        return np.ascontiguousarray(a.reshape(2, 128, n).transpose(1, 0, 2))

    def flat(a):
        return np.ascontiguousarray(a).reshape(128, -1)

    # packed per-dtype const blocks, one DMA each (offsets in _build)
    constsAf = np.concatenate([
        flat(kchunked(c12).astype(f32)),             # 128 w
        flat(kchunked(c3).astype(f32)),              # 512 w
        flat(np.eye(128, dtype=f32)),                # 128 w
        flat(vecs),                                  # 12 w
    ], axis=1)
    constsAb = np.concatenate([
        flat(kchunked(co).astype(bf16)),             # 512 e
        flat(lhsT_w1.astype(bf16)),                  # 128 e
        flat(lhsT_w2.astype(bf16)),                  # 128 e
        flat(np.eye(128, dtype=bf16)),               # 128 e
    ], axis=1)
    # 8-partition consts: [8, 128 + 9*784] bf16 = (lhsT_pos | posr)
    constsB = np.concatenate([lhsT_pos.astype(bf16), posr.astype(bf16)],
                             axis=1)
    return {"constsAf": constsAf, "constsAb": constsAb, "constsB": constsB}


# ------------------------------------------------------------ program build
def _build_program():
    nc = bacc.Bacc("TRN2", target_bir_lowering=False, debug=False,
                   enable_asserts=False, num_devices=8)

    def dram_in(name, shape, dtype):
        return nc.dram_tensor(name, list(shape), dtype, kind="ExternalInput").ap()

    xin = dram_in("xin", (CIN, NPIX), F32R)
    cAfd = dram_in("constsAf", (128, 780), F32R)
    cAbd = dram_in("constsAb", (128, 896), BF16)
    cBd = dram_in("constsB", (8, 128 + 9 * Q), BF16)

    outd = nc.dram_tensor("out", [CIN, NPIX], F32, kind="ExternalOutput").ap()

    with tile.TileContext(nc) as tc, ExitStack() as ctx:
        nc_ = tc.nc
        sdma = nc.sync.dma_start            # HW DGE queue (SP). The Pool
                                            # queue's SW DGE burns ~1us/DMA
                                            # of GPSIMD engine time.

        cpool = ctx.enter_context(tc.tile_pool(name="consts", bufs=1))
        xpool = ctx.enter_context(tc.tile_pool(name="xres", bufs=1))
        bpool = ctx.enter_context(tc.tile_pool(name="bands", bufs=1))
        fpool = ctx.enter_context(tc.tile_pool(name="fpool", bufs=4))
        hpool = ctx.enter_context(tc.tile_pool(name="hpool", bufs=2))
        epool = ctx.enter_context(tc.tile_pool(name="epool", bufs=9))
        zpool = ctx.enter_context(tc.tile_pool(name="zpool", bufs=1))
        pkpool = ctx.enter_context(tc.tile_pool(name="pkpool", bufs=12))
        rqpool = ctx.enter_context(tc.tile_pool(name="rqpool", bufs=2))
        sqpool = ctx.enter_context(tc.tile_pool(name="sqpool", bufs=2))
        smcpool = ctx.enter_context(tc.tile_pool(name="smcpool", bufs=1))
        opool = ctx.enter_context(tc.tile_pool(name="opool", bufs=4))

        # ---- const loads: three packed DMAs split across both HW DGE
        # queues (sync + scalar) so the front-door transfers run in parallel
        adma = nc.scalar.dma_start
        cAf = cpool.tile([128, 780], F32R, tag="cAf")
        sdma(cAf[:], cAfd[:])
        cAb = cpool.tile([128, 896], BF16, tag="cAb")
        adma(cAb[:], cAbd[:])
        cB = cpool.tile([8, 128 + 9 * Q], BF16, tag="cB")
        adma(cB[:], cBd[:])
        c12t = cAf[:, 0:128].rearrange("p (a b) -> p a b", a=2)
        c3t = cAf[:, 128:640].rearrange("p (a b) -> p a b", a=2)
        identft = cAf[:, 640:768]
        vecst = cAf[:, 768:780].bitcast(F32)
        cot = cAb[:, 0:512].rearrange("p (a b) -> p a b", a=2)
        w1t = cAb[:, 512:640]
        w2t = cAb[:, 640:768]
        identt = cAb[:, 768:896]
        post = cB[:, 0:128]
        prts = [cB[:, 128 + Q * k:128 + Q * (k + 1)] for k in range(9)]

        # ---- x resident in SBUF, loaded in row groups (sync queue)
        xt = [xpool.tile([128, NPIX], F32R, tag=f"xt{t}", name=f"xt{t}")
              for t in range(2)]
        for (rl, rh) in ((0, 16), (16, 30), (30, 44), (44, 56)):
            sdma(xt[0][:, rl * W:rh * W], xin[0:128, rl * W:rh * W])
            adma(xt[1][:, rl * W:rh * W], xin[128:256, rl * W:rh * W])

        # ---- band tiles + halo memsets
        x12s = bpool.tile([64, NPIX], BF16, tag="x12s")
        x1b = bpool.tile([128, BH, W], BF16, tag="x1b")
        x2b = bpool.tile([128, BR, 58], BF16, tag="x2b")
        xgA = bpool.tile([128, SHARE, BR, 58], BF16, tag="xgA")
        xgB = bpool.tile([128, SHARE, BR, 58], BF16, tag="xgB")
        nc_.gpsimd.memset(x2b[:], 0.0)
        for xg_ in (xgA, xgB):
            nc_.gpsimd.memset(xg_[0:32, :, 0:1, :], 0.0)
            nc_.gpsimd.memset(xg_[96:128, :, 15:16, :], 0.0)

        # ---- conv1+conv2: 7 chunks, ACT moves bias into x12s
        pscopeA = ExitStack()
        pp12 = pscopeA.enter_context(tc.tile_pool(name="pp12", bufs=3, space="PSUM"))
        for c in range(NCH):
            sl = slice(CCH * c, CCH * (c + 1))
            ps = pp12.tile([64, CCH], F32, tag="ps12")
            nc_.tensor.matmul(ps[:], c12t[:, 0, :], xt[0][:, sl],
                              start=True, stop=False)
            nc_.tensor.matmul(ps[:], c12t[:, 1, :], xt[1][:, sl],
                              start=False, stop=True)
            nc_.scalar.activation(x12s[:, sl], ps[:], ACTF.Identity,
                                  bias=vecst[0:64, 0:1])
        pscopeA.close()

        # ---- x1/x2 band scatter (SBUF->SBUF, vector queue: self-gating
        # with the DVE feat ops that consume them)
        x1bf = x1b[:].rearrange("p r w -> p (r w)")
        for b in range(NB):
            lo, hi = _band_rows(b)
            r0 = lo - (14 * b - 1)
            sdma(x1bf[32 * b:32 * b + 32, :], x12s[0:32, Q * b:Q * (b + 1)])
            sdma(x2b[32 * b:32 * b + 32, r0:r0 + (hi - lo), 1:57],
                 x12s[32:64, lo * W:hi * W].rearrange("p (r w) -> p r w", w=W))

        # ---- conv3 -> per-t staging tile -> xgA/xgB band scatter
        x3a0 = bpool.tile([128, H, 58], BF16, tag="x3a0")
        x3a1 = bpool.tile([128, H, 58], BF16, tag="x3a1")
        x3all = [x3a0, x3a1]
        for t in range(2):
            nc_.gpsimd.memset(x3all[t][:, :, 0:1], 0.0)
            nc_.gpsimd.memset(x3all[t][:, :, 57:58], 0.0)
        # band b is fully staged once chunk BLAST[b] is written
        BLAST = {1: 0, 3: 1, 5: 2, 6: 3}
        pscopeB = ExitStack()
        pp3 = pscopeB.enter_context(tc.tile_pool(name="pp3", bufs=3, space="PSUM"))
        for c in range(NCH):
            for t in range(2):
                ps3 = pp3.tile([128, CCH], F32, tag="ps3")
                nc_.tensor.matmul(ps3[:], c3t[:, 0, 128 * t:128 * (t + 1)],
                                  xt[0][:, CCH * c:CCH * (c + 1)],
                                  start=True, stop=False)
                nc_.tensor.matmul(ps3[:], c3t[:, 1, 128 * t:128 * (t + 1)],
                                  xt[1][:, CCH * c:CCH * (c + 1)],
                                  start=False, stop=True)
                nc_.scalar.activation(x3all[t][:, 8 * c:8 * c + 8, 1:57],
                                      ps3[:].rearrange("p (r w) -> p r w", w=W),
                                      ACTF.Identity, bias=vecst[:, 1 + t:2 + t])
            if c in BLAST:
                b = BLAST[c]
                lo, hi = _band_rows(b)
                dr0 = lo - (14 * b - 1)
                nrr = hi - lo
                for t in range(2):
                    x3f = x3all[t][:].rearrange("p r w -> p (r w)")
                    psl = slice(32 * b + 16 * t, 32 * b + 16 * t + 16)
                    dstA = xgA[psl].rearrange("p s r w -> p s (r w)")
                    sdma(dstA[:, :, dr0 * 58:(dr0 + nrr) * 58],
                         x3f[:, lo * 58:hi * 58])
                    # xgB[w] = xgA[w+1]: shifted flat copy, 1 desc/partition
                    dstB = xgB[psl].rearrange("p s r w -> p s (r w)")
                    sdma(dstB[:, :, dr0 * 58:(dr0 + nrr) * 58 - 1],
                         x3f[:, lo * 58 + 1:hi * 58])
        pscopeB.close()

        # ---- phase C: per-k logits -> exp; Z accumulated on GPSIMD.
        # Software-pipelined one k deep: w2(k-1)/e(k-1) are emitted after
        # w1(k)/hp(k) so the PE queue never stalls on the ACT relu.
        wsl = ((0, 512), (512, 272))
        pscopeC = ExitStack()
        pph = pscopeC.enter_context(tc.tile_pool(name="pph", bufs=3, space="PSUM"))
        ek = []
        hpk = {}
        zs = [zpool.tile([128, Q], F32, tag=f"z{i}", name=f"z{i}")
              for i in range(2)]
        zcur = None

        def emit_w2_e(kk):
            nonlocal zcur
            wps = pph.tile([128, 1024], F32, tag="hw", name="wps")
            for (o0, nn) in wsl:
                nc_.tensor.matmul(wps[:, o0:o0 + nn], w2t[:],
                                  hpk[kk][:, o0:o0 + nn], start=True, stop=True)
            e = epool.tile([128, Q], BF16, tag="e", name="e")
            nc_.scalar.activation(e[:], wps[:, 0:Q], ACTF.Exp,
                                  bias=vecst[:, 4:5])
            ek.append(e)
            if kk == 1:
                nc_.gpsimd.tensor_tensor(zs[0][:], ek[0][:], ek[1][:], ALU.add)
                zcur = 0
            elif kk >= 2:
                nc_.gpsimd.tensor_tensor(zs[1 - zcur][:], e[:], zs[zcur][:],
                                         ALU.add)
                zcur = 1 - zcur

        for k, (dh, dw) in enumerate(OFFS):
            win = x2b[:, 1 + dh:15 + dh, 1 + dw:57 + dw]
            ftmp = fpool.tile([128, BH, W], BF16, tag="ftmp")
            if dw == 0:
                nc_.gpsimd.tensor_tensor(ftmp[:], x1b[:], win, ALU.subtract)
            else:
                nc_.vector.tensor_tensor(ftmp[:], x1b[:], win, ALU.subtract)
            fr = fpool.tile([128, BH, W], BF16, tag="fr")
            nc_.vector.tensor_scalar(fr[:], ftmp[:], 0.0, None, op0=ALU.max)
            frf = fr[:].rearrange("p r w -> p (r w)")
            hps = pph.tile([128, 1024], F32, tag="hw", name="hps")
            for (o0, nn) in wsl:
                nc_.tensor.matmul(hps[:, o0:o0 + nn], w1t[:], frf[:, o0:o0 + nn],
                                  start=True, stop=False)
                nc_.tensor.matmul(hps[:, o0:o0 + nn], post[:],
                                  prts[k][:, o0:o0 + nn], start=False, stop=True)
            hp = hpool.tile([128, Q], BF16, tag="hp", name="hp")
            nc_.scalar.activation(hp[:], hps[:, 0:Q], ACTF.Relu,
                                  bias=vecst[:, 3:4])
            hpk[k] = hp
            if k >= 1:
                emit_w2_e(k - 1)
        emit_w2_e(8)
        pscopeC.close()

        # ---- aggregation rounds (qp: row half, sq: s-quad), software-
        # pipelined: round r+1's DVE products are emitted before round r's
        # post so the PE identity stream never stalls at round boundaries.
        pscopeE = ExitStack()
        pps = pscopeE.enter_context(tc.tile_pool(name="pps", bufs=1, space="PSUM"))
        ppo = pscopeE.enter_context(tc.tile_pool(name="ppo", bufs=2, space="PSUM"))
        korder = [2, 3, 4, 5, 6, 7, 8, 0, 1]
        rounds = [(0, 0), (0, 1), (1, 0), (1, 1)]
        smcs = {}
        for qp in range(2):
            for bb in range(NB):
                for t in range(2):
                    smcs[(qp, bb, t)] = smcpool.tile(
                        [128, HBW], BF16, tag=f"smc{qp}_{bb}_{t}",
                        name=f"smc{qp}_{bb}_{t}")
        pks = {}
        sams = {}

        def emit_product(ri, k, on_gpsimd=False):
            qp, sq = rounds[ri]
            dh, dw = OFFS[k]
            xgt = xgB if dw == 0 else xgA
            co_ = 0 if dw == 0 else 1 + dw
            r0 = 1 + dh + 7 * qp
            pk = pkpool.tile([128, 4, 7, W], BF16, tag="pk", name="pk")
            win = xgt[:, 4 * sq:4 * sq + 4, r0:r0 + 7, co_:co_ + W]
            e3 = (ek[k][:].rearrange("p (r w) -> p r w", w=W)
                  [:, 7 * qp:7 * qp + 7, :])
            if on_gpsimd:
                # Pool TT is limited to 3D operands: per-s'-lane ops
                for a in range(4):
                    nc_.gpsimd.tensor_tensor(pk[:, a, :, :], win[:, a, :, :],
                                             e3, ALU.mult)
            else:
                ein = e3.unsqueeze(1).broadcast_to((128, 4, 7, W))
                nc_.vector.tensor_tensor(pk[:], win, ein, ALU.mult)
            pks[(ri, k)] = pk

        def emit_mms(ri, extra_pe=()):
            # extra_pe: convo units interleaved into the identity stream to
            # fill the PE while the DVE feeds products
            sam = pps.tile([128, 2048], F32, tag="sam", name="sam")
            sams[ri] = sam
            extra = list(extra_pe)
            for j, k in enumerate(korder):
                pkf = pks[(ri, k)][:].rearrange("p a r w -> p (a r w)")
                for a in range(4):
                    nc_.tensor.matmul(
                        sam[:, 512 * a:512 * a + HBW], identt[:],
                        pkf[:, HBW * a:HBW * (a + 1)],
                        start=(j == 0), stop=(j == 8))
                if j >= 2 and extra:
                    emit_convo_unit(*extra.pop(0))
            for u in extra:
                emit_convo_unit(*u)

        def emit_post(ri):
            qp, sq = rounds[ri]
            sam = sams[ri]
            rq = rqpool.tile([128, 4, HBW], BF16, tag="rq", name="rq")
            sq_ = sqpool.tile([128, 4, HBW], BF16, tag="sq", name="sq_")
            for a in range(4):
                nc_.scalar.activation(rq[:, a, :],
                                      sam[:, 512 * a:512 * a + HBW],
                                      ACTF.Relu, scale=-(1.0 - NEG))
                nc_.vector.scalar_tensor_tensor(
                    sq_[:, a, :], sam[:, 512 * a:512 * a + HBW], 1.0,
                    rq[:, a, :], ALU.mult, ALU.add)
                nc_.vector.tensor_tensor(
                    sq_[:, a, :], sq_[:, a, :],
                    rz16[:, HBW * qp:HBW * (qp + 1)], ALU.mult)
            # scatter to conv_out rhs layout (SBUF->SBUF): smc partition
            # 64sq+4gl+a <- src partition 32bb+16t+gl lane a (the convo
            # lhsT rows are host-permuted to match)
            for bb in range(NB):
                for t in range(2):
                    p0 = 32 * bb + 16 * t
                    sdma(smcs[(qp, bb, t)][64 * sq:64 * sq + 64, :],
                         sq_[p0:p0 + 16, :, :])

        def emit_convo_unit(qp, bb, tp):
            # conv_out + leaky + residual; the leaky correction
            # r = relu(-0.99(p+b)) and the residual x are identity-
            # accumulated into PSUM so DVE stays out of the tail:
            # out = p + x + r + b = leaky(p + b) + x
            po = (14 * bb + 7 * qp) * W
            pso = ppo.tile([128, HBW], F32, tag="pso", name="pso")
            nc_.tensor.matmul(pso[:], cot[:, 0, 128 * tp:128 * (tp + 1)],
                              smcs[(qp, bb, 0)][:], start=True, stop=False)
            nc_.tensor.matmul(pso[:], cot[:, 1, 128 * tp:128 * (tp + 1)],
                              smcs[(qp, bb, 1)][:], start=False, stop=True)
            r = opool.tile([128, HBW], BF16, tag="r", name="r")
            nc_.scalar.activation(r[:], pso[:], ACTF.Relu,
                                  scale=-(1.0 - NEG),
                                  bias=vecst[:, 5 + tp:6 + tp])
            # keep accumulating into the (hw-wise still live) psum:
            # stop above only closes the sim's group bookkeeping
            nc_.tensor.matmul(pso[:], identft[:], xt[tp][:, po:po + HBW],
                              start=False, stop=False, skip_group_check=True)
            nc_.tensor.matmul(pso[:], identt[:], r[:],
                              start=False, stop=True, skip_group_check=True)
            o = opool.tile([128, HBW], F32, tag="o", name="o")
            nc_.scalar.activation(o[:], pso[:], ACTF.Identity,
                                  bias=vecst[:, 7 + tp:8 + tp])
            sdma(outd[128 * tp:128 * (tp + 1), po:po + HBW], o[:])

        def emit_round(ri, convo_units=()):
            # gpsimd gets k0/k1 (consumed last by the PE); the first two DVE
            # products land before the previous round's post so the PE can
            # enter this round almost immediately
            emit_product(ri, 0, on_gpsimd=True)
            emit_product(ri, 1, on_gpsimd=True)
            emit_product(ri, 2)
            emit_product(ri, 3)
            if ri > 0:
                emit_post(ri - 1)
            for k in (4, 5, 6, 7, 8):
                emit_product(ri, k)
            if ri == 0:
                # 1/Z (applied after leaky: positively homogeneous)
                nc_.vector.reciprocal_approx_fast(zs[1 - zcur][:], zs[zcur][:])
                nc_.vector.tensor_copy(rz16[:], zs[1 - zcur][:])
            emit_mms(ri, extra_pe=convo_units)

        rz16 = zpool.tile([128, Q], BF16, tag="rz16")
        u0 = [(0, bb, tp) for bb in range(NB) for tp in range(2)]
        u1 = [(1, bb, tp) for bb in range(NB) for tp in range(2)]
        emit_round(0)
        emit_round(1)
        emit_round(2, convo_units=u0[:4])
        emit_round(3, convo_units=u0[4:])
        emit_post(3)
        for u in u1:
            emit_convo_unit(*u)
        pscopeE.close()

    nc.compile()
    return nc


# --------------------------------------------------------------- entrypoint
def _get_program():
    if "nc" not in _CACHE:
        _CACHE["nc"] = _build_program()
    return _CACHE["nc"]


def _in_maps(inputs):
    consts = _host_consts(inputs)
    x = inputs["x"].reshape(B, CIN, NPIX).astype(np.float32)
    in_maps = []
    for b in range(B):
        m = {k: v for k, v in consts.items()}
        m["xin"] = x[b]
        in_maps.append(m)
    return in_maps


def kernel(**inputs):
    inputs = {k: np.asarray(v) for k, v in inputs.items()}
    nc = _get_program()
    res = run_bass_kernel_spmd(nc, _in_maps(inputs), list(range(B)))
    out = np.stack([res.results[i]["out"] for i in range(B)])
    return out.reshape(B, CIN, H, W).astype(np.float32)


def kernel_traced(**inputs):
    """Like kernel() but with NTFF tracing; returns (out, BassKernelResults)."""
    inputs = {k: np.asarray(v) for k, v in inputs.items()}
    nc = _get_program()
    res = run_bass_kernel_spmd(nc, _in_maps(inputs), list(range(B)), trace=True)
    out = np.stack([res.results[i]["out"] for i in range(B)])
    return out.reshape(B, CIN, H, W).astype(np.float32), res


# revision 46
# speedup vs baseline: 1.0023x; 1.0022x over previous
"""Trainium2 Bass kernel for nn_Bottleneck_57561151701110 (SAM pairwise
bottleneck block). Data-parallel over batch: 8 images -> 8 NeuronCores.

Per-core pipeline (one 256x56x56 image, x kept resident in SBUF):
  conv1+conv2 (PE f32r) -> ACT bias move -> SBUF DMA band scatter ->
  pairwise feat = x1 - shift(x2) (DVE bf16 2x / GPSIMD for the odd-parity
  dw=0 shifts) -> w1/w2 blockdiag matmuls (PE bf16) + ACT relu/exp ->
  Z = sum_k e_k on GPSIMD, 1/Z via fast-approx reciprocal (DVE) ->
  aggregation: 9 broadcast multiplies (DVE bf16 2x, 2/round on GPSIMD)
  + PE identity-accumulate into PSUM -> leaky + 1/Z scale -> SBUF DMA
  channel scatter -> conv_out (PE bf16) -> leaky + residual -> out.

BN layers are folded into conv weights/biases on the host. The position
branch enters the w1 matmul as an extra low-rank accumulate. All DMA
triggers issue from the GPSIMD queue (cheap dispatch); x1/x2/x3/sam
partition reshuffles are SBUF->SBUF DMAs (no DRAM roundtrips).
"""

import os
import sys

for _p in ("/opt/trn_rl_repo", os.path.expanduser("~/.axon_site/_ro/trn_rl_repo")):
    if os.path.isdir(_p) and _p not in sys.path:
        sys.path.insert(0, _p)

from contextlib import ExitStack

import numpy as np

import concourse.bass as bass
import concourse.bacc as bacc
import concourse.tile as tile
from concourse import mybir
from concourse.bass_utils import run_bass_kernel_spmd

dt = mybir.dt
ALU = mybir.AluOpType
ACTF = mybir.ActivationFunctionType

B, CIN, H, W = 8, 256, 56, 56
NPIX = H * W            # 3136
REL, MID, OUT = 32, 256, 256
SHARE = 8
NB = 4                  # row bands
BH = H // NB            # 14 rows per band
Q = BH * W              # 784 band pixels
BR = BH + 2             # 16 padded band rows
HBW = Q // 2            # 392 half-band pixels
NEG = 0.01
BN_EPS = 1e-5
CCH = 448               # conv free chunk (8 rows)
NCH = NPIX // CCH       # 7
OFFS = [(dh, dw) for dh in (-1, 0, 1) for dw in (-1, 0, 1)]

F32, F32R, BF16 = dt.float32, dt.float32r, dt.float16  # 16-bit = fp16

_CACHE = {}


def _band_rows(b):
    # image-row window covered by band b's x2/x3 tiles (halo included)
    return max(14 * b - 1, 0), min(14 * b + 15, 56)


# ----------------------------------------------------------------- host prep
def _position(h, w):
    loc_w = np.tile(np.linspace(-1.0, 1.0, w, dtype=np.float32)[None, :], (h, 1))
    loc_h = np.tile(np.linspace(-1.0, 1.0, h, dtype=np.float32)[:, None], (1, w))
    return np.stack([loc_w, loc_h], axis=0)  # (2, H, W)


def _host_consts(inp):
    f32 = np.float32
    inv_a = (inp["bna_g"] / np.sqrt(inp["bna_v"] + BN_EPS)).astype(f32)
    beta_a = (inp["bna_b"] - inp["bna_m"] * inv_a).astype(f32)
    inv_b = (inp["bnb_g"] / np.sqrt(inp["bnb_v"] + BN_EPS)).astype(f32)
    beta_b = (inp["bnb_b"] - inp["bnb_m"] * inv_b).astype(f32)

    w1c = inp["conv1_w"] * inv_a[:REL, None]
    b1 = inp["conv1_b"] * inv_a[:REL] + beta_a[:REL]
    w2c = inp["conv2_w"] * inv_a[:REL, None]
    b2 = inp["conv2_b"] * inv_a[:REL]

    # conv1+conv2 fused stationary: (256, 64), chunked over K
    c12 = np.concatenate([w1c, w2c], axis=0).T.astype(f32).copy()  # (256,64)

    c3 = inp["conv3_w"].T.astype(f32).copy()    # (256,256) lhsT
    # conv_out lhsT with contraction rows permuted to the aggregation's
    # scatter-friendly order: row 128t+r holds input channel
    # 8*(16t + gl) + 4*sq + a where r = (sq, gl, a) = (r>>6, (r&63)>>2, r&3)
    co_n = inp["convo_w"].T.astype(f32)         # (256 in, 256 out)
    co = np.zeros_like(co_n)
    for t in range(2):
        for r in range(128):
            sq, gl, a = r >> 6, (r & 63) >> 2, r & 3
            co[128 * t + r] = co_n[8 * (16 * t + gl) + 4 * sq + a]

    # W1' with bnb scale folded
    w1p = (inp["w1"] * inv_b[:, None]).astype(f32)  # (32, 34)
    w1a, w1b = w1p[:, :REL], w1p[:, REL:]
    lhsT_w1 = np.zeros((128, 128), f32)
    lhsT_pos = np.zeros((8, 128), f32)
    lhsT_w2 = np.zeros((128, 128), f32)
    for b in range(NB):
        lhsT_w1[32 * b:32 * b + 32, 32 * b:32 * b + 32] = w1a.T  # [c, o]
        lhsT_pos[2 * b:2 * b + 2, 32 * b:32 * b + 32] = w1b.T    # [c2, o]
        lhsT_w2[32 * b:32 * b + 32, 32 * b:32 * b + 32] = inp["w2"].T  # [o, g]

    betab = np.tile(beta_b, NB).astype(f32).reshape(128)
    w2bv = np.tile(inp["w2_b"], NB).astype(f32).reshape(128)

    # position branch, batch independent: posr[2b+c2, 784k+q] =
    # relu(inv_a[32+c2]*subp[c2,k,band b pix q] + beta_a[32+c2])
    pos = _position(H, W)
    pc = np.einsum("oc,chw->ohw", inp["convp_w"], pos) + inp["convp_b"][:, None, None]
    pcp = np.pad(pc, ((0, 0), (1, 1), (1, 1)))
    posr = np.zeros((8, 9 * Q), f32)
    for k, (dh, dw) in enumerate(OFFS):
        sub = pc - pcp[:, 1 + dh:1 + dh + H, 1 + dw:1 + dw + W]  # (2,56,56)
        v = np.maximum(inv_a[REL:, None, None] * sub + beta_a[REL:, None, None], 0.0)
        vb = v.reshape(2, NB, BH, W)  # (c2, b, r, w)
        for b in range(NB):
            posr[2 * b:2 * b + 2, Q * k:Q * (k + 1)] = vb[:, b].reshape(2, Q)

    vecs = np.zeros((128, 12), f32)
    vecs[:REL, 0] = b1
    vecs[REL:2 * REL, 0] = b2
    vecs[:, 1] = inp["conv3_b"][:128]
    vecs[:, 2] = inp["conv3_b"][128:]
    vecs[:, 3] = betab
    vecs[:, 4] = w2bv
    vecs[:, 5] = -(1.0 - NEG) * inp["convo_b"][:128]
    vecs[:, 6] = -(1.0 - NEG) * inp["convo_b"][128:]
    vecs[:, 7] = inp["convo_b"][:128]
    vecs[:, 8] = inp["convo_b"][128:]

    bf16 = np.float16

    def kchunked(a):
        # (256, N) lhsT -> [128, 2, N]: partition p holds rows p and 128+p
        n = a.shape[1]
        return np.ascontiguousarray(a.reshape(2, 128, n).transpose(1, 0, 2))

    def flat(a):
        return np.ascontiguousarray(a).reshape(128, -1)

    # packed per-dtype const blocks, one DMA each (offsets in _build)
    constsAf = np.concatenate([
        flat(kchunked(c12).astype(f32)),             # 128 w
        flat(kchunked(c3).astype(f32)),              # 512 w
        flat(np.eye(128, dtype=f32)),                # 128 w
        flat(vecs),                                  # 12 w
    ], axis=1)
    constsAb = np.concatenate([
        flat(kchunked(co).astype(bf16)),             # 512 e
        flat(lhsT_w1.astype(bf16)),                  # 128 e
        flat(lhsT_w2.astype(bf16)),                  # 128 e
        flat(np.eye(128, dtype=bf16)),               # 128 e
    ], axis=1)
    # 8-partition consts: [8, 128 + 9*784] bf16 = (lhsT_pos | posr)
    constsB = np.concatenate([lhsT_pos.astype(bf16), posr.astype(bf16)],
                             axis=1)
    return {"constsAf": constsAf, "constsAb": constsAb, "constsB": constsB}


# ------------------------------------------------------------ program build
def _build_program():
    nc = bacc.Bacc("TRN2", target_bir_lowering=False, debug=False,
                   enable_asserts=False, num_devices=8)

    def dram_in(name, shape, dtype):
        return nc.dram_tensor(name, list(shape), dtype, kind="ExternalInput").ap()

    xin = dram_in("xin", (CIN, NPIX), F32R)
    cAfd = dram_in("constsAf", (128, 780), F32R)
    cAbd = dram_in("constsAb", (128, 896), BF16)
    cBd = dram_in("constsB", (8, 128 + 9 * Q), BF16)

    outd = nc.dram_tensor("out", [CIN, NPIX], F32, kind="ExternalOutput").ap()

    with tile.TileContext(nc) as tc, ExitStack() as ctx:
        nc_ = tc.nc
        sdma = nc.sync.dma_start            # HW DGE queue (SP). The Pool
                                            # queue's SW DGE burns ~1us/DMA
                                            # of GPSIMD engine time.

        cpool = ctx.enter_context(tc.tile_pool(name="consts", bufs=1))
        xpool = ctx.enter_context(tc.tile_pool(name="xres", bufs=1))
        bpool = ctx.enter_context(tc.tile_pool(name="bands", bufs=1))
        fpool = ctx.enter_context(tc.tile_pool(name="fpool", bufs=2))
        hpool = ctx.enter_context(tc.tile_pool(name="hpool", bufs=2))
        epool = ctx.enter_context(tc.tile_pool(name="epool", bufs=9))
        zpool = ctx.enter_context(tc.tile_pool(name="zpool", bufs=1))
        pkpool = ctx.enter_context(tc.tile_pool(name="pkpool", bufs=9))
        rqpool = ctx.enter_context(tc.tile_pool(name="rqpool", bufs=1))
        sqpool = ctx.enter_context(tc.tile_pool(name="sqpool", bufs=2))
        smcpool = ctx.enter_context(tc.tile_pool(name="smcpool", bufs=1))
        opool = ctx.enter_context(tc.tile_pool(name="opool", bufs=2))

        # ---- const loads: three packed DMAs split across both HW DGE
        # queues (sync + scalar) so the front-door transfers run in parallel
        adma = nc.scalar.dma_start
        cAf = cpool.tile([128, 780], F32R, tag="cAf")
        sdma(cAf[:], cAfd[:])
        cAb = cpool.tile([128, 896], BF16, tag="cAb")
        adma(cAb[:], cAbd[:])
        cB = cpool.tile([8, 128 + 9 * Q], BF16, tag="cB")
        adma(cB[:], cBd[:])
        c12t = cAf[:, 0:128].rearrange("p (a b) -> p a b", a=2)
        c3t = cAf[:, 128:640].rearrange("p (a b) -> p a b", a=2)
        identft = cAf[:, 640:768]
        vecst = cAf[:, 768:780].bitcast(F32)
        cot = cAb[:, 0:512].rearrange("p (a b) -> p a b", a=2)
        w1t = cAb[:, 512:640]
        w2t = cAb[:, 640:768]
        identt = cAb[:, 768:896]
        post = cB[:, 0:128]
        prts = [cB[:, 128 + Q * k:128 + Q * (k + 1)] for k in range(9)]

        # ---- x resident in SBUF, loaded in row groups (sync queue)
        xt = [xpool.tile([128, NPIX], F32R, tag=f"xt{t}", name=f"xt{t}")
              for t in range(2)]
        for (rl, rh) in ((0, 16), (16, 30), (30, 44), (44, 56)):
            sdma(xt[0][:, rl * W:rh * W], xin[0:128, rl * W:rh * W])
            adma(xt[1][:, rl * W:rh * W], xin[128:256, rl * W:rh * W])

        # ---- band tiles + halo memsets
        x12s = bpool.tile([64, NPIX], BF16, tag="x12s")
        x1b = bpool.tile([128, BH, W], BF16, tag="x1b")
        x2b = bpool.tile([128, BR, 58], BF16, tag="x2b")
        xgA = bpool.tile([128, SHARE, BR, 58], BF16, tag="xgA")
        xgB = bpool.tile([128, SHARE, BR, 58], BF16, tag="xgB")
        nc_.gpsimd.memset(x2b[:], 0.0)
        for xg_ in (xgA, xgB):
            nc_.gpsimd.memset(xg_[0:32, :, 0:1, :], 0.0)
            nc_.gpsimd.memset(xg_[96:128, :, 15:16, :], 0.0)

        # ---- conv1+conv2: 7 chunks, ACT moves bias into x12s
        pscopeA = ExitStack()
        pp12 = pscopeA.enter_context(tc.tile_pool(name="pp12", bufs=3, space="PSUM"))
        for c in range(NCH):
            sl = slice(CCH * c, CCH * (c + 1))
            ps = pp12.tile([64, CCH], F32, tag="ps12")
            nc_.tensor.matmul(ps[:], c12t[:, 0, :], xt[0][:, sl],
                              start=True, stop=False)
            nc_.tensor.matmul(ps[:], c12t[:, 1, :], xt[1][:, sl],
                              start=False, stop=True)
            nc_.scalar.activation(x12s[:, sl], ps[:], ACTF.Identity,
                                  bias=vecst[0:64, 0:1])
        pscopeA.close()

        # ---- x1/x2 band scatter (SBUF->SBUF, vector queue: self-gating
        # with the DVE feat ops that consume them)
        x1bf = x1b[:].rearrange("p r w -> p (r w)")
        for b in range(NB):
            lo, hi = _band_rows(b)
            r0 = lo - (14 * b - 1)
            sdma(x1bf[32 * b:32 * b + 32, :], x12s[0:32, Q * b:Q * (b + 1)])
            sdma(x2b[32 * b:32 * b + 32, r0:r0 + (hi - lo), 1:57],
                 x12s[32:64, lo * W:hi * W].rearrange("p (r w) -> p r w", w=W))

        # ---- conv3 -> per-t staging tile -> xgA/xgB band scatter
        x3a0 = bpool.tile([128, H, 58], BF16, tag="x3a0")
        x3a1 = bpool.tile([128, H, 58], BF16, tag="x3a1")
        x3all = [x3a0, x3a1]
        for t in range(2):
            nc_.gpsimd.memset(x3all[t][:, :, 0:1], 0.0)
            nc_.gpsimd.memset(x3all[t][:, :, 57:58], 0.0)
        # band b is fully staged once chunk BLAST[b] is written
        BLAST = {1: 0, 3: 1, 5: 2, 6: 3}
        pscopeB = ExitStack()
        pp3 = pscopeB.enter_context(tc.tile_pool(name="pp3", bufs=3, space="PSUM"))
        for c in range(NCH):
            for t in range(2):
                ps3 = pp3.tile([128, CCH], F32, tag="ps3")
                nc_.tensor.matmul(ps3[:], c3t[:, 0, 128 * t:128 * (t + 1)],
                                  xt[0][:, CCH * c:CCH * (c + 1)],
                                  start=True, stop=False)
                nc_.tensor.matmul(ps3[:], c3t[:, 1, 128 * t:128 * (t + 1)],
                                  xt[1][:, CCH * c:CCH * (c + 1)],
                                  start=False, stop=True)
                nc_.scalar.activation(x3all[t][:, 8 * c:8 * c + 8, 1:57],
                                      ps3[:].rearrange("p (r w) -> p r w", w=W),
                                      ACTF.Identity, bias=vecst[:, 1 + t:2 + t])
            if c in BLAST:
                b = BLAST[c]
                lo, hi = _band_rows(b)
                dr0 = lo - (14 * b - 1)
                nrr = hi - lo
                for t in range(2):
                    x3f = x3all[t][:].rearrange("p r w -> p (r w)")
                    psl = slice(32 * b + 16 * t, 32 * b + 16 * t + 16)
                    dstA = xgA[psl].rearrange("p s r w -> p s (r w)")
                    sdma(dstA[:, :, dr0 * 58:(dr0 + nrr) * 58],
                         x3f[:, lo * 58:hi * 58])
                    # xgB[w] = xgA[w+1]: shifted flat copy, 1 desc/partition
                    dstB = xgB[psl].rearrange("p s r w -> p s (r w)")
                    sdma(dstB[:, :, dr0 * 58:(dr0 + nrr) * 58 - 1],
                         x3f[:, lo * 58 + 1:hi * 58])
        pscopeB.close()

        # ---- phase C: per-k logits -> exp; Z accumulated on GPSIMD.
        # Software-pipelined one k deep: w2(k-1)/e(k-1) are emitted after
        # w1(k)/hp(k) so the PE queue never stalls on the ACT relu.
        wsl = ((0, 512), (512, 272))
        pscopeC = ExitStack()
        pph = pscopeC.enter_context(tc.tile_pool(name="pph", bufs=3, space="PSUM"))
        ek = []
        hpk = {}
        zs = [zpool.tile([128, Q], F32, tag=f"z{i}", name=f"z{i}")
              for i in range(2)]
        zcur = None

        def emit_w2_e(kk):
            nonlocal zcur
            wps = pph.tile([128, 1024], F32, tag="hw", name="wps")
            for (o0, nn) in wsl:
                nc_.tensor.matmul(wps[:, o0:o0 + nn], w2t[:],
                                  hpk[kk][:, o0:o0 + nn], start=True, stop=True)
            e = epool.tile([128, Q], BF16, tag="e", name="e")
            nc_.scalar.activation(e[:], wps[:, 0:Q], ACTF.Exp,
                                  bias=vecst[:, 4:5])
            ek.append(e)
            if kk == 1:
                nc_.gpsimd.tensor_tensor(zs[0][:], ek[0][:], ek[1][:], ALU.add)
                zcur = 0
            elif kk >= 2:
                nc_.gpsimd.tensor_tensor(zs[1 - zcur][:], e[:], zs[zcur][:],
                                         ALU.add)
                zcur = 1 - zcur

        for k, (dh, dw) in enumerate(OFFS):
            win = x2b[:, 1 + dh:15 + dh, 1 + dw:57 + dw]
            ftmp = fpool.tile([128, BH, W], BF16, tag="ftmp")
            if dw == 0:
                nc_.gpsimd.tensor_tensor(ftmp[:], x1b[:], win, ALU.subtract)
            else:
                nc_.vector.tensor_tensor(ftmp[:], x1b[:], win, ALU.subtract)
            fr = fpool.tile([128, BH, W], BF16, tag="fr")
            nc_.vector.tensor_scalar(fr[:], ftmp[:], 0.0, None, op0=ALU.max)
            frf = fr[:].rearrange("p r w -> p (r w)")
            hps = pph.tile([128, 1024], F32, tag="hw", name="hps")
            for (o0, nn) in wsl:
                nc_.tensor.matmul(hps[:, o0:o0 + nn], w1t[:], frf[:, o0:o0 + nn],
                                  start=True, stop=False)
                nc_.tensor.matmul(hps[:, o0:o0 + nn], post[:],
                                  prts[k][:, o0:o0 + nn], start=False, stop=True)
            hp = hpool.tile([128, Q], BF16, tag="hp", name="hp")
            nc_.scalar.activation(hp[:], hps[:, 0:Q], ACTF.Relu,
                                  bias=vecst[:, 3:4])
            hpk[k] = hp
            if k >= 1:
                emit_w2_e(k - 1)
        emit_w2_e(8)
        pscopeC.close()

        # ---- aggregation rounds (qp: row half, sq: s-quad), software-
        # pipelined: round r+1's DVE products are emitted before round r's
        # post so the PE identity stream never stalls at round boundaries.
        pscopeE = ExitStack()
        pps = pscopeE.enter_context(tc.tile_pool(name="pps", bufs=1, space="PSUM"))
        ppo = pscopeE.enter_context(tc.tile_pool(name="ppo", bufs=2, space="PSUM"))
        korder = [2, 3, 4, 5, 6, 7, 8, 0, 1]
        # paired order: both row-halves of an s-quad share one set of
        # full-height products
        rounds = [(0, 0), (1, 0), (1, 1), (0, 1)]
        smcs = {}
        for qp in range(2):
            for bb in range(NB):
                for t in range(2):
                    smcs[(qp, bb, t)] = smcpool.tile(
                        [128, HBW], BF16, tag=f"smc{qp}_{bb}_{t}",
                        name=f"smc{qp}_{bb}_{t}")
        pks = {}
        sams = {}

        def emit_product(sq, k):
            # full-height product (both row halves), shared by the qp pair
            dh, dw = OFFS[k]
            xgt = xgB if dw == 0 else xgA
            co_ = 0 if dw == 0 else 1 + dw
            pk = pkpool.tile([128, 4, BH, W], BF16, tag="pk", name="pk")
            win = xgt[:, 4 * sq:4 * sq + 4, 1 + dh:1 + dh + BH, co_:co_ + W]
            ein = (ek[k][:].rearrange("p (r w) -> p r w", w=W)
                   .unsqueeze(1).broadcast_to((128, 4, BH, W)))
            nc_.vector.tensor_tensor(pk[:], win, ein, ALU.mult)
            pks[(sq, k)] = pk

        def emit_mms(ri, extra_pe=()):
            # extra_pe: convo units interleaved into the identity stream to
            # fill the PE while the DVE feeds products
            qp, sq = rounds[ri]
            sam = pps.tile([128, 2048], F32, tag="sam", name="sam")
            sams[ri] = sam
            extra = list(extra_pe)
            for j, k in enumerate(korder):
                pkt = pks[(sq, k)]
                for a in range(4):
                    rhs = (pkt[:, a, 7 * qp:7 * qp + 7, :]
                           .rearrange("p r w -> p (r w)"))
                    nc_.tensor.matmul(
                        sam[:, 512 * a:512 * a + HBW], identt[:], rhs,
                        start=(j == 0), stop=(j == 8))
                if j >= 2 and extra:
                    emit_convo_unit(*extra.pop(0))
            for u in extra:
                emit_convo_unit(*u)

        def emit_post(ri):
            qp, sq = rounds[ri]
            sam = sams[ri]
            rq = rqpool.tile([128, 4, HBW], BF16, tag="rq", name="rq")
            sq_ = sqpool.tile([128, 4, HBW], BF16, tag="sq", name="sq_")
            for a in range(4):
                nc_.scalar.activation(rq[:, a, :],
                                      sam[:, 512 * a:512 * a + HBW],
                                      ACTF.Relu, scale=-(1.0 - NEG))
                nc_.vector.scalar_tensor_tensor(
                    sq_[:, a, :], sam[:, 512 * a:512 * a + HBW], 1.0,
                    rq[:, a, :], ALU.mult, ALU.add)
                nc_.vector.tensor_tensor(
                    sq_[:, a, :], sq_[:, a, :],
                    rz16[:, HBW * qp:HBW * (qp + 1)], ALU.mult)
            # scatter to conv_out rhs layout (SBUF->SBUF): smc partition
            # 64sq+4gl+a <- src partition 32bb+16t+gl lane a (the convo
            # lhsT rows are host-permuted to match)
            for bb in range(NB):
                for t in range(2):
                    p0 = 32 * bb + 16 * t
                    sdma(smcs[(qp, bb, t)][64 * sq:64 * sq + 64, :],
                         sq_[p0:p0 + 16, :, :])

        def emit_convo_unit(qp, bb, tp):
            # conv_out + leaky + residual; the leaky correction
            # r = relu(-0.99(p+b)) and the residual x are identity-
            # accumulated into PSUM so DVE stays out of the tail:
            # out = p + x + r + b = leaky(p + b) + x
            po = (14 * bb + 7 * qp) * W
            pso = ppo.tile([128, HBW], F32, tag="pso", name="pso")
            nc_.tensor.matmul(pso[:], cot[:, 0, 128 * tp:128 * (tp + 1)],
                              smcs[(qp, bb, 0)][:], start=True, stop=False)
            nc_.tensor.matmul(pso[:], cot[:, 1, 128 * tp:128 * (tp + 1)],
                              smcs[(qp, bb, 1)][:], start=False, stop=True)
            r = opool.tile([128, HBW], BF16, tag="r", name="r")
            nc_.scalar.activation(r[:], pso[:], ACTF.Relu,
                                  scale=-(1.0 - NEG),
                                  bias=vecst[:, 5 + tp:6 + tp])
            # keep accumulating into the (hw-wise still live) psum:
            # stop above only closes the sim's group bookkeeping
            nc_.tensor.matmul(pso[:], identft[:], xt[tp][:, po:po + HBW],
                              start=False, stop=False, skip_group_check=True)
            nc_.tensor.matmul(pso[:], identt[:], r[:],
                              start=False, stop=True, skip_group_check=True)
            o = opool.tile([128, HBW], F32, tag="o", name="o")
            nc_.scalar.activation(o[:], pso[:], ACTF.Identity,
                                  bias=vecst[:, 7 + tp:8 + tp])
            sdma(outd[128 * tp:128 * (tp + 1), po:po + HBW], o[:])

        rz16 = zpool.tile([128, Q], BF16, tag="rz16")
        u0 = [(0, bb, tp) for bb in range(NB) for tp in range(2)]
        u1 = [(1, bb, tp) for bb in range(NB) for tp in range(2)]
        # pair 0 (sq=0): rounds r0=(qp0) / r1=(qp1) share products
        for k in korder:
            emit_product(0, k)
        # 1/Z (applied after leaky: positively homogeneous)
        nc_.vector.reciprocal_approx_fast(zs[1 - zcur][:], zs[zcur][:])
        nc_.vector.tensor_copy(rz16[:], zs[1 - zcur][:])
        emit_mms(0)
        emit_post(0)
        emit_mms(1)
        # pair 1 (sq=1): products staged around post(r1)
        emit_product(1, 2)
        emit_product(1, 3)
        emit_post(1)
        for k in (4, 5, 6, 7, 8, 0, 1):
            emit_product(1, k)
        emit_mms(2)
        emit_post(2)
        emit_mms(3, extra_pe=u1)
        emit_post(3)
        for u in u0:
            emit_convo_unit(*u)
        pscopeE.close()

    nc.compile()
    return nc


# --------------------------------------------------------------- entrypoint
def _get_program():
    if "nc" not in _CACHE:
        _CACHE["nc"] = _build_program()
    return _CACHE["nc"]


def _in_maps(inputs):
    consts = _host_consts(inputs)
    x = inputs["x"].reshape(B, CIN, NPIX).astype(np.float32)
    in_maps = []
    for b in range(B):
        m = {k: v for k, v in consts.items()}
        m["xin"] = x[b]
        in_maps.append(m)
    return in_maps


def kernel(**inputs):
    inputs = {k: np.asarray(v) for k, v in inputs.items()}
    nc = _get_program()
    res = run_bass_kernel_spmd(nc, _in_maps(inputs), list(range(B)))
    out = np.stack([res.results[i]["out"] for i in range(B)])
    return out.reshape(B, CIN, H, W).astype(np.float32)


def kernel_traced(**inputs):
    """Like kernel() but with NTFF tracing; returns (out, BassKernelResults)."""
    inputs = {k: np.asarray(v) for k, v in inputs.items()}
    nc = _get_program()
    res = run_bass_kernel_spmd(nc, _in_maps(inputs), list(range(B)), trace=True)
    out = np.stack([res.results[i]["out"] for i in range(B)])
    return out.reshape(B, CIN, H, W).astype(np.float32), res
